# revision 8
# baseline (speedup 1.0000x reference)
"""Single-head attention (B=8, T=2048, D=E=1024, fp32) with attention-prob
dropout, for 8 Trainium2 NeuronCores.

Sharding: data-parallel over batch — core b computes batch element b.

All big matmuls run on TensorE in float32r (4-byte fp32 with reduced
mantissa, 1 cyc/row at moving-dim>=256 — 4x faster than plain fp32).
The BIR verifier requires every SBUF operand of an fp32r matmul to be
*written* as fp32r (the producing engine rounds), so all matmul operand
tiles/scratch are allocated with that dtype and filled by compute
engines or by DMA from fp32r scratch.

Per-core pipeline:
  Phase B: transpose x -> xT (PE identity transpose), project
           qT[E,T], kT[E,T] (transposed layouts) and v[T,E] (natural),
           spill all three to DRAM scratch (fp32r).
  Phase C: flash-style over t-tiles of 512 columns:
           S^T[s,t] = kT.T @ qT  (PSUM), exp on ScalarE (D^-1/2 scale
           fused), per-column sums via PE matmuls against a ones
           vector (gives per-partition colsum^T directly), dropout mask
           multiply on VectorE, P^T @ v accumulation, and a final
           PSUM->SBUF copy on ScalarE with the 1/colsum normalization
           fused as a per-partition activation scale.

The dropout keep-mask is input-independent (fixed key 42), so it is
computed once on host CPU with jax (bit-identical threefry to the
reference), pre-transposed/scaled to maskT[s,t] = keep[t,s]/0.8 in bf16
(0 and 1.25 are exact), and streamed as an extra kernel input.
"""

import os
import sys

for _p in ("/opt/trn_rl_repo",):
    if _p not in sys.path:
        sys.path.append(_p)

import numpy as np

import concourse.bass as bass  # noqa: F401
import concourse.tile as tile
from concourse import bacc, mybir
from concourse import bass_utils
from concourse.masks import make_identity

P = 128
B, T, D, E = 8, 2048, 1024, 1024
# debug-only override (grading always uses the full sizes)
T = int(os.environ.get("KERNEL_DEBUG_T", T))
DC, EC, SC = D // P, E // P, T // P
SLAB = 512          # phase-B t-slab
NSLAB = T // SLAB
TT = 512            # phase-C t-tile
NTT = T // TT
TSUB = TT // P      # 4
EHN = E // 512      # 2
SCALE = float(D) ** -0.5  # 1/32
KEEP_P = 0.8

F32 = mybir.dt.float32
F32R = mybir.dt.float32r
BF16 = mybir.dt.bfloat16

_PROGRAM = None
_MASKT = None


def _build_program():
    nc = bacc.Bacc(
        "TRN2",
        target_bir_lowering=False,
        debug=False,
        enable_asserts=False,
        num_devices=8,
    )

    x_d = nc.dram_tensor("x", [T, D], F32, kind="ExternalInput")
    wq_d = nc.dram_tensor("Wq", [D, E], F32, kind="ExternalInput")
    wk_d = nc.dram_tensor("Wk", [D, E], F32, kind="ExternalInput")
    wv_d = nc.dram_tensor("Wv", [D, E], F32, kind="ExternalInput")
    maskT_d = nc.dram_tensor("maskT", [T, T], BF16, kind="ExternalInput")
    out_d = nc.dram_tensor("out", [T, E], F32, kind="ExternalOutput")

    # DRAM scratch (spills), rounded to fp32r by the phase-B copies.
    # qT[ec, p, t] = q[t, ec*128+p]; kT[sc, ec, p, s] = k[sc*128+s, ec*128+p]
    qT_s = nc.dram_tensor("qT_scratch", [EC, P, T], F32R, kind="Internal")
    kT_s = nc.dram_tensor("kT_scratch", [SC, EC, P, P], F32R, kind="Internal")
    v_s = nc.dram_tensor("v_scratch", [T, E], F32R, kind="Internal")

    x_ap = x_d.ap()
    maskT_ap = maskT_d.ap()
    out_ap = out_d.ap()
    qT_ap = qT_s.ap()
    kT_ap = kT_s.ap()
    v_ap = v_s.ap()

    Exp = mybir.ActivationFunctionType.Exp
    Copy = mybir.ActivationFunctionType.Copy
    Mult = mybir.AluOpType.mult

    # Alternate PSUM->SBUF copies between VectorE and ScalarE.
    _cp = [0]

    def copy_ps(nc, dst, src):
        if _cp[0] % 2 == 0:
            nc.vector.tensor_copy(dst, src)
        else:
            nc.scalar.copy(dst, src)
        _cp[0] += 1

    with tile.TileContext(nc) as tc:
        with tc.tile_pool(name="const", bufs=1) as const:
            ident = const.tile([P, P], F32)
            make_identity(nc, ident[:])
            ones_f = const.tile([P, 2], F32)
            nc.gpsimd.memset(ones_f[:], 1.0)
            # fp32r matmuls need even innermost counts, so the ones vector
            # (and the colsum dst) are 2 columns wide; both get the sum.
            ones = const.tile([P, 2], F32R)
            nc.vector.tensor_copy(ones[:], ones_f[:])  # round to fp32r

            # ---------------- Phase B: xT, projections, spill ----------------
            with (
                tc.tile_pool(name="w", bufs=1) as wpool,
                tc.tile_pool(name="wtmp", bufs=3) as wtmp_pool,
                tc.tile_pool(name="xin", bufs=2) as xin_pool,
                tc.tile_pool(name="xT", bufs=2) as xT_pool,
                tc.tile_pool(name="pout", bufs=6) as pout_pool,
                tc.tile_pool(name="ps_tr", bufs=2, space="PSUM") as ps_tr,
                tc.tile_pool(name="ps_proj", bufs=2, space="PSUM") as ps_proj,
            ):
                w_sbs = []
                for wi, w_d in enumerate((wq_d, wk_d, wv_d)):
                    w_sb = wpool.tile([P, DC, E], F32R, tag=f"w{wi}")
                    wap = w_d.ap().rearrange("(dc p) e -> p dc e", p=P)
                    for dc in range(DC):
                        wt = wtmp_pool.tile([P, E], F32, tag="wtmp")
                        nc.sync.dma_start(wt[:], wap[:, dc, :])
                        copy_ps(nc, w_sb[:, dc, :], wt[:])  # fp32 -> fp32r
                    w_sbs.append(w_sb)
                wq_sb, wk_sb, wv_sb = w_sbs

                for slab in range(NSLAB):
                    t0 = slab * SLAB
                    xs = xin_pool.tile([P, SLAB // P, D], F32, tag="xin")
                    nc.sync.dma_start(
                        xs[:],
                        x_ap[t0 : t0 + SLAB, :].rearrange(
                            "(ts p) d -> p ts d", p=P
                        ),
                    )
                    xT = xT_pool.tile([P, DC, SLAB], F32R, tag="xT")
                    for ts_i in range(SLAB // P):
                        for dc in range(DC):
                            pt = ps_tr.tile([P, P], F32)
                            nc.tensor.transpose(
                                pt[:], xs[:, ts_i, dc * P : (dc + 1) * P], ident[:]
                            )
                            copy_ps(
                                nc, xT[:, dc, ts_i * P : (ts_i + 1) * P], pt[:]
                            )

                    # qT / kT: psum[e-chunk(128), t(512)] = Wx[:,dc,ec]^T @ xT
                    for w_sb, is_q in ((wq_sb, True), (wk_sb, False)):
                        for ec in range(EC):
                            pp = ps_proj.tile([P, SLAB], F32, tag="pp")
                            for dc in range(DC):
                                nc.tensor.matmul(
                                    pp[:],
                                    w_sb[:, dc, ec * P : (ec + 1) * P],
                                    xT[:, dc, :],
                                    start=(dc == 0),
                                    stop=(dc == DC - 1),
                                )
                            ob = pout_pool.tile([P, SLAB], F32R, tag="pout")
                            copy_ps(nc, ob[:], pp[:])
                            if is_q:
                                nc.sync.dma_start(
                                    qT_ap[ec, :, t0 : t0 + SLAB], ob[:]
                                )
                            else:
                                for i in range(SLAB // P):
                                    nc.sync.dma_start(
                                        kT_ap[slab * (SLAB // P) + i, ec],
                                        ob[:, i * P : (i + 1) * P],
                                    )

                    # v: psum[t-sub(128), e(512)] = xT[:,dc,tsub]^T @ Wv
                    for ts_i in range(SLAB // P):
                        for eh in range(EHN):
                            pp = ps_proj.tile([P, 512], F32, tag="pp")
                            for dc in range(DC):
                                nc.tensor.matmul(
                                    pp[:],
                                    xT[:, dc, ts_i * P : (ts_i + 1) * P],
                                    wv_sb[:, dc, eh * 512 : (eh + 1) * 512],
                                    start=(dc == 0),
                                    stop=(dc == DC - 1),
                                )
                            ob = pout_pool.tile([P, 512], F32R, tag="pout")
                            copy_ps(nc, ob[:], pp[:])
                            r0 = t0 + ts_i * P
                            nc.sync.dma_start(
                                v_ap[r0 : r0 + P, eh * 512 : (eh + 1) * 512],
                                ob[:],
                            )

            # ---------------- Phase C: attention ----------------
            with (
                tc.tile_pool(name="v", bufs=1) as vpool,
                tc.tile_pool(name="qTt", bufs=2) as qt_pool,
                tc.tile_pool(name="kTc", bufs=3) as kt_pool,
                tc.tile_pool(name="mask", bufs=SC + 2) as mask_pool,
                tc.tile_pool(name="PT", bufs=2) as pt_pool,
                tc.tile_pool(name="osb", bufs=4) as osb_pool,
                tc.tile_pool(name="small", bufs=2) as small_pool,
                tc.tile_pool(name="ps_S", bufs=2, space="PSUM") as ps_S,
                tc.tile_pool(name="ps_cs", bufs=2, space="PSUM") as ps_cs,
                tc.tile_pool(name="ps_out", bufs=2, space="PSUM") as ps_out,
            ):
                v_sb = vpool.tile([P, SC, E], F32R)
                for sc in range(SC):
                    nc.sync.dma_start(
                        v_sb[:, sc, :], v_ap[sc * P : (sc + 1) * P, :]
                    )

                for tt in range(NTT):
                    c0 = tt * TT
                    qTt = qt_pool.tile([P, EC, TT], F32R, tag="qTt")
                    nc.sync.dma_start(
                        qTt[:],
                        qT_ap[:, :, c0 : c0 + TT].rearrange("ec p t -> p ec t"),
                    )
                    PT = pt_pool.tile([P, SC, TT], F32R, tag="PT")
                    mks = []
                    for sc in range(SC):
                        kTc = kt_pool.tile([P, EC, P], F32R, tag="kTc")
                        nc.sync.dma_start(
                            kTc[:], kT_ap[sc].rearrange("ec p s -> p ec s")
                        )
                        mk = mask_pool.tile([P, TT], BF16, tag="mask")
                        nc.sync.dma_start(
                            mk[:], maskT_ap[sc * P : (sc + 1) * P, c0 : c0 + TT]
                        )
                        mks.append(mk)
                        ps = ps_S.tile([P, TT], F32, tag="ps")
                        for ec in range(EC):
                            nc.tensor.matmul(
                                ps[:],
                                kTc[:, ec, :],
                                qTt[:, ec, :],
                                start=(ec == 0),
                                stop=(ec == EC - 1),
                            )
                        nc.scalar.activation(
                            PT[:, sc, :], ps[:], Exp, scale=SCALE
                        )

                    # colsum^T[t,1] per t-sub via ones-matmuls (pre-mask).
                    # One accumulation group (= one PSUM tile) per t-sub.
                    recip = small_pool.tile([P, TSUB], F32, tag="recip")
                    for j in range(TSUB):
                        cs = ps_cs.tile([P, 2], F32, tag="cs")
                        for sc in range(SC):
                            nc.tensor.matmul(
                                cs[:],
                                PT[:, sc, j * P : (j + 1) * P],
                                ones[:],
                                start=(sc == 0),
                                stop=(sc == SC - 1),
                            )
                        nc.vector.reciprocal(recip[:, j : j + 1], cs[:, 0:1])
                    # dropout mask (in place; WAR on colsum reads)
                    for sc in range(SC):
                        nc.vector.tensor_tensor(
                            PT[:, sc, :], PT[:, sc, :], mks[sc][:], Mult
                        )

                    for j in range(TSUB):
                        for eh in range(EHN):
                            po = ps_out.tile([P, 512], F32, tag="po")
                            for sc in range(SC):
                                nc.tensor.matmul(
                                    po[:],
                                    PT[:, sc, j * P : (j + 1) * P],
                                    v_sb[:, sc, eh * 512 : (eh + 1) * 512],
                                    start=(sc == 0),
                                    stop=(sc == SC - 1),
                                )
                            ob = osb_pool.tile([P, 512], F32, tag="osb")
                            nc.scalar.activation(
                                ob[:], po[:], Copy, scale=recip[:, j : j + 1]
                            )
                            r0 = c0 + j * P
                            nc.sync.dma_start(
                                out_ap[r0 : r0 + P, eh * 512 : (eh + 1) * 512],
                                ob[:],
                            )

    nc.compile()
    return nc


def get_program():
    global _PROGRAM
    if _PROGRAM is None:
        _PROGRAM = _build_program()
    return _PROGRAM


def get_maskT():
    """keep[t,s]/keep_p, transposed to [s,t], bf16, per batch element."""
    global _MASKT
    if _MASKT is None:
        import jax
        import ml_dtypes

        with jax.default_device(jax.devices("cpu")[0]):
            keep = jax.random.bernoulli(
                jax.random.key(42), KEEP_P, (B, T, T)
            )
            keep = np.asarray(keep)
        maskT = np.transpose(keep, (0, 2, 1)).astype(ml_dtypes.bfloat16)
        maskT = maskT * np.asarray(1.0 / KEEP_P, dtype=ml_dtypes.bfloat16)
        _MASKT = np.ascontiguousarray(maskT)
    return _MASKT


def kernel(x, Wq, Wk, Wv):
    x = np.ascontiguousarray(np.asarray(x, dtype=np.float32))
    Wq = np.ascontiguousarray(np.asarray(Wq, dtype=np.float32))
    Wk = np.ascontiguousarray(np.asarray(Wk, dtype=np.float32))
    Wv = np.ascontiguousarray(np.asarray(Wv, dtype=np.float32))
    maskT = get_maskT()
    nc = get_program()
    in_maps = [
        {"x": x[b], "Wq": Wq, "Wk": Wk, "Wv": Wv, "maskT": maskT[b]}
        for b in range(B)
    ]
    res = bass_utils.run_bass_kernel_spmd(nc, in_maps, core_ids=list(range(B)))
    return np.stack([res.results[b]["out"] for b in range(B)], axis=0)
